# revision 1
# baseline (speedup 1.0000x reference)
"""Single-head causal attention (B=4, T=2048, D=1024, H=64) on 8 TRN2 cores.

Sharding: core = (batch b, group g). Each core owns the interleaved half of
the query blocks of one batch (g=0: even 128-row blocks, g=1: odd), arranged
"mine first, partner second" via a host-side row permutation so the causal
structure (and hence the instruction stream) is identical on all 8 cores.

Numerics: score path (projections + q@k^T) uses bf16 hi/lo split operands
with 3 matmul passes (~18-bit effective mantissa, fp32 PSUM accumulation);
probs and V use plain bf16. Softmax uses exact per-row max (masked before
max), exp on ACT with fused row-sum, normalization folded into the final
output copy. probs and v are transposed with the DMA xbar (bf16).
"""

import numpy as np
import ml_dtypes

import concourse.bass as bass
import concourse.bacc as bacc
import concourse.tile as tile
import concourse.mybir as mybir
from concourse.bass_utils import run_bass_kernel_spmd
from concourse.masks import make_identity

BF16 = mybir.dt.bfloat16
F32 = mybir.dt.float32
BFNP = ml_dtypes.bfloat16

B, T, D, H, P = 4, 2048, 1024, 64, 128
NB = T // P       # 16 key tiles (128 rows each) per batch
NQ = NB // 2      # 8 local query blocks per core
DC = D // P       # 8 contraction chunks
KC = 512          # matmul moving-dim chunk (one PSUM bank of fp32)
NEG = -1.0e30
ACT = mybir.ActivationFunctionType


def _bank_chunks(c0, c1):
    """Split [c0, c1) into <=512-wide pieces that don't cross 512 boundaries."""
    out = []
    while c0 < c1:
        nxt = min(c1, (c0 // KC + 1) * KC)
        out.append((c0, nxt))
        c0 = nxt
    return out


def build_nc():
    nc = bacc.Bacc("TRN2", target_bir_lowering=False, debug=False,
                   num_devices=8)
    xt_hi = nc.dram_tensor("xt_hi", [D, T], BF16, kind="ExternalInput")
    xt_lo = nc.dram_tensor("xt_lo", [D, T], BF16, kind="ExternalInput")
    # weights pre-arranged on host to the SBUF layout [P, DC*H]
    wq_hi = nc.dram_tensor("wq_hi", [P, DC * H], BF16, kind="ExternalInput")
    wq_lo = nc.dram_tensor("wq_lo", [P, DC * H], BF16, kind="ExternalInput")
    wk_hi = nc.dram_tensor("wk_hi", [P, DC * H], BF16, kind="ExternalInput")
    wk_lo = nc.dram_tensor("wk_lo", [P, DC * H], BF16, kind="ExternalInput")
    wv = nc.dram_tensor("wv", [P, DC * H], BF16, kind="ExternalInput")
    bq8 = nc.dram_tensor("bq8", [H, 1], F32, kind="ExternalInput")
    bkv = nc.dram_tensor("bkv", [H, 2], F32, kind="ExternalInput")
    maska = nc.dram_tensor("maska", [P, P], F32, kind="ExternalInput")
    maskb = nc.dram_tensor("maskb", [P, P], F32, kind="ExternalInput")
    out = nc.dram_tensor("out", [NQ * P, H], F32, kind="ExternalOutput")

    with tile.TileContext(nc) as tc:
        with (
            tc.tile_pool(name="singles", bufs=1) as singles,
            tc.tile_pool(name="probs", bufs=3) as probs_pool,
            tc.tile_pool(name="stats", bufs=6) as stats,
        ):
            # ---- persistent SBUF ----
            s_xth = singles.tile([P, DC, T], BF16)
            s_xtl = singles.tile([P, DC, T], BF16)
            s_wqh = singles.tile([P, DC, H], BF16)
            s_wql = singles.tile([P, DC, H], BF16)
            s_wkh = singles.tile([P, DC, H], BF16)
            s_wkl = singles.tile([P, DC, H], BF16)
            s_wv = singles.tile([P, DC, H], BF16)
            s_bq8 = singles.tile([H, 1], F32)
            s_bkv = singles.tile([H, 2], F32)
            s_ma = singles.tile([P, P], F32)
            s_mb = singles.tile([P, P], F32)

            # hi chunks on the SP queue, lo chunks on the ACT queue, in
            # consumption order so chunk-pair c arrives while c-1 computes.
            for s_w, d_w in ((s_wv, wv), (s_wqh, wq_hi), (s_wql, wq_lo),
                             (s_wkh, wk_hi), (s_wkl, wk_lo)):
                nc.scalar.dma_start(
                    s_w[:, :, :].rearrange("p c h -> p (c h)"), d_w[:, :])
            nc.scalar.dma_start(s_bq8[:, :], bq8[:, :])
            nc.scalar.dma_start(s_bkv[:, :], bkv[:, :])
            nc.scalar.dma_start(s_ma[:, :], maska[:, :])
            nc.scalar.dma_start(s_mb[:, :], maskb[:, :])
            for c in range(DC):
                nc.sync.dma_start(s_xth[:, c, :], xt_hi[c * P:(c + 1) * P, :])
                nc.scalar.dma_start(s_xtl[:, c, :],
                                    xt_lo[c * P:(c + 1) * P, :])

            s_v = singles.tile([H, T], BF16)
            s_vnat = singles.tile([P, NB, H], BF16)
            s_qf = singles.tile([H, NQ * P], F32)
            s_qh = singles.tile([H, NQ * P], BF16)
            s_ql = singles.tile([H, NQ * P], BF16)
            s_kf = singles.tile([H, T], F32)
            s_kh = singles.tile([H, T], BF16)
            s_kl = singles.tile([H, T], BF16)
            s_probsT = singles.tile([P, NB, NQ * P], BF16)
            s_z = singles.tile([P, NQ, 4], F32)
            nc.gpsimd.memset(s_z[:, :, :], 0.0)
            s_outT = singles.tile([H, NQ * P], F32)

            # ---- phase 1: projections (vT, then qT/kT bf16x2 3-pass) ----
            with tc.tile_pool(name="pj_psum", bufs=1, space="PSUM") as pj:
                vps = pj.tile([H, T], F32, tag="big")
                for c in range(DC):
                    for n0 in range(0, T, KC):
                        nc.tensor.matmul(
                            vps[:, n0:n0 + KC], lhsT=s_wv[:, c, :],
                            rhs=s_xth[:, c, n0:n0 + KC],
                            start=(c == 0), stop=(c == DC - 1))
                # qT/kT/vT c-outer so chunk-pair c streams in while c-1
                # computes; the last c iteration is peeled per 512-col region
                # with the bf16 hi/lo re-split fused right behind it.
                qps = pj.tile([H, NQ * P], F32, tag="small")
                kps = pj.tile([H, T], F32, tag="big")
                passes = ((s_wqh, s_wkh, s_xth), (s_wql, s_wkl, s_xth),
                          (s_wqh, s_wkh, s_xtl))
                for c in range(DC):
                    for n0 in range(0, T, KC):
                        nc.tensor.matmul(
                            vps[:, n0:n0 + KC], lhsT=s_wv[:, c, :],
                            rhs=s_xth[:, c, n0:n0 + KC],
                            start=(c == 0), stop=(c == DC - 1))
                    if c == DC - 1:
                        break
                    for ip, (wq_pl, wk_pl, xpl) in enumerate(passes):
                        for n0 in range(0, NQ * P, KC):
                            nc.tensor.matmul(
                                qps[:, n0:n0 + KC], lhsT=wq_pl[:, c, :],
                                rhs=xpl[:, c, n0:n0 + KC],
                                start=(c == 0 and ip == 0), stop=False)
                        for n0 in range(0, T, KC):
                            nc.tensor.matmul(
                                kps[:, n0:n0 + KC], lhsT=wk_pl[:, c, :],
                                rhs=xpl[:, c, n0:n0 + KC],
                                start=(c == 0 and ip == 0), stop=False)

                nc.scalar.copy(s_v[:, :], vps[:, :])
                nc.sync.dma_start(s_vnat[:, :, :], s_v[:, :], transpose=True)

                # peeled c=7 per region + immediate split, ordered so block 7
                # (q region 1, all k regions) unblocks earliest
                c = DC - 1
                for which, n0 in (("q", KC), ("k", 0), ("k", KC),
                                  ("k", 2 * KC), ("k", 3 * KC), ("q", 0)):
                    ps, pf, ph, pl, bias, scl = (
                        (qps, s_qf, s_qh, s_ql, s_bq8[:, 0:1], 8.0)
                        if which == "q" else
                        (kps, s_kf, s_kh, s_kl, s_bkv[:, 0:1], 1.0))
                    for ip, (wq_pl, wk_pl, xpl) in enumerate(passes):
                        wpl = wq_pl if which == "q" else wk_pl
                        nc.tensor.matmul(
                            ps[:, n0:n0 + KC], lhsT=wpl[:, c, :],
                            rhs=xpl[:, c, n0:n0 + KC],
                            start=False, stop=(ip == 2))
                    nc.scalar.activation(pf[:, n0:n0 + KC], ps[:, n0:n0 + KC],
                                         ACT.Identity, bias=bias, scale=scl)
                    nc.scalar.copy(ph[:, n0:n0 + KC], pf[:, n0:n0 + KC])
                    nc.vector.tensor_tensor(
                        pl[:, n0:n0 + KC], pf[:, n0:n0 + KC],
                        ph[:, n0:n0 + KC], mybir.AluOpType.subtract)

            # ---- phase 2: attention blocks (descending) + interleaved AV ----
            with (
                tc.tile_pool(name="sc_psum", bufs=6, space="PSUM") as sc,
                tc.tile_pool(name="av_psum", bufs=1, space="PSUM") as avp,
            ):
                avps = avp.tile([H, NQ * P], F32, tag="av")
                av_banks_started = set()
                spass = ((s_qh, s_kh), (s_qh, s_kl), (s_ql, s_kh))
                for i in range(NQ - 1, -1, -1):
                    K = P * (i + 1)
                    q0, q1 = i * P, (i + 1) * P
                    nch = (K + KC - 1) // KC  # chunks per part (1 or 2)
                    mx4 = stats.tile([P, 4], F32, tag="mx")
                    chunks = []  # (sc_tile, psum_slice_len, ci)
                    for pi, (koff, msk) in enumerate(((0, s_ma),
                                                      (NQ * P, s_mb))):
                        for n0 in range(0, K, KC):
                            nn = min(KC, K - n0)
                            sp = sc.tile([P, KC], F32, tag="sc")
                            for ip, (qpl, kpl) in enumerate(spass):
                                nc.tensor.matmul(
                                    sp[:, 0:nn], lhsT=qpl[:, q0:q1],
                                    rhs=kpl[:, koff + n0:koff + n0 + nn],
                                    start=(ip == 0), stop=(ip == 2))
                            if n0 + nn == K:  # mask rides in the last chunk
                                nc.vector.tensor_tensor(
                                    sp[:, nn - P:nn], sp[:, nn - P:nn],
                                    msk[:, :], mybir.AluOpType.add)
                            ci = pi * nch + n0 // KC
                            nc.vector.tensor_reduce(
                                mx4[:, ci:ci + 1], sp[:, 0:nn],
                                axis=mybir.AxisListType.X,
                                op=mybir.AluOpType.max)
                            chunks.append((sp, pi, n0, nn, ci))
                    negm = stats.tile([P, 1], F32, tag="negm")
                    nc.vector.tensor_reduce(negm[:, :], mx4[:, 0:2 * nch],
                                            axis=mybir.AxisListType.X,
                                            op=mybir.AluOpType.max,
                                            negate=True)
                    # exp per chunk (frees score PSUM早), then one batched
                    # transpose per part: A on the SP HWDGE queue, B on ACT
                    probs = probs_pool.tile([P, T], BF16)
                    for sp, pi, n0, nn, ci in chunks:
                        po = pi * K + n0
                        nc.scalar.activation(probs[:, po:po + nn], sp[:, 0:nn],
                                             ACT.Exp, bias=negm[:, :],
                                             scale=1.0,
                                             accum_out=s_z[:, i, ci:ci + 1])
                    nc.sync.dma_start(s_probsT[:, 0:i + 1, q0:q1],
                                      probs[:, 0:K], transpose=True)
                    nc.scalar.dma_start(s_probsT[:, NQ:NQ + i + 1, q0:q1],
                                        probs[:, K:2 * K], transpose=True)
                # AV tail: outT[h, q] += v_nat[t].T @ probsT[t]
                for t in range(NB):
                    j = t % NQ
                    c0 = j * P
                    if t < NQ:
                        chunks_av = [(a, b, False)
                                     for a, b in _bank_chunks(c0, NQ * P)]
                    else:
                        chunks_av = [(c0, c0 + P, True)]
                        chunks_av += [(a, b, False)
                                      for a, b in _bank_chunks(c0 + P, NQ * P)]
                    for a, b, stp in chunks_av:
                        nc.tensor.matmul(
                            avps[:, a:b], lhsT=s_vnat[:, t, :],
                            rhs=s_probsT[:, t, a:b], start=(t == 0), stop=stp)

                # per-block +bv copy out of PSUM so the finalize pipelines
                for j in range(NQ):
                    nc.scalar.activation(s_outT[:, j * P:(j + 1) * P],
                                         avps[:, j * P:(j + 1) * P],
                                         ACT.Identity, bias=s_bkv[:, 1:2],
                                         scale=1.0)

            # ---- phase 3: +bv, transpose to [q, H], divide by Z, store ----
            with tc.tile_pool(name="tr_psum", bufs=2, space="PSUM") as trp:
                s_id = singles.tile([H, H], F32)
                make_identity(nc, s_id[:, :])
                s_zs = singles.tile([P, NQ], F32)
                nc.vector.tensor_reduce(s_zs[:, :], s_z[:, :, :],
                                        axis=mybir.AxisListType.X,
                                        op=mybir.AluOpType.add)
                s_rz = singles.tile([P, NQ], F32)
                nc.vector.reciprocal(s_rz[:, :], s_zs[:, :])
                s_on = singles.tile([P, NQ, H], F32)
                out3 = out[:, :].rearrange("(j p) h -> j p h", p=P)
                for j in range(NQ):
                    tps = trp.tile([P, H], F32, tag="tr")
                    nc.tensor.transpose(tps[:, :],
                                        s_outT[:, j * P:(j + 1) * P],
                                        s_id[:, :])
                    nc.vector.tensor_scalar_mul(s_on[:, j, :], tps[:, :],
                                                s_rz[:, j:j + 1])
                    nc.sync.dma_start(out3[j, :, :], s_on[:, j, :])
    nc.compile()
    return nc


_NC_CACHE = {}


def _get_nc():
    if "nc" not in _NC_CACHE:
        _NC_CACHE["nc"] = build_nc()
    return _NC_CACHE["nc"]


def _split_bf(a):
    hi = a.astype(BFNP)
    lo = (a - hi.astype(np.float32)).astype(BFNP)
    return hi, lo


LAST_RESULT = None


def kernel(x, Wq, bq, Wk, bk, Wv, bv, _trace=False, **_run_kwargs):
    global LAST_RESULT
    x = np.ascontiguousarray(np.asarray(x, dtype=np.float32))
    Wq = np.asarray(Wq, dtype=np.float32)
    Wk = np.asarray(Wk, dtype=np.float32)
    Wv = np.asarray(Wv, dtype=np.float32)
    bq = np.asarray(bq, dtype=np.float32)
    bk = np.asarray(bk, dtype=np.float32)
    bv = np.asarray(bv, dtype=np.float32)

    def _w_layout(w):
        # [D, H] -> SBUF layout [P, DC*H] (chunk-major along free dim)
        return np.ascontiguousarray(
            w.reshape(DC, P, H).transpose(1, 0, 2).reshape(P, DC * H))

    wqh, wql = _split_bf(Wq)
    wkh, wkl = _split_bf(Wk)
    wqh, wql = _w_layout(wqh), _w_layout(wql)
    wkh, wkl = _w_layout(wkh), _w_layout(wkl)
    wvh = _w_layout(Wv.astype(BFNP))
    bq8 = np.ascontiguousarray((8.0 * bq).reshape(H, 1))
    bkv = np.ascontiguousarray(np.stack([bk, bv], axis=1))  # [H, 2]
    r = np.arange(P)
    maska = np.where(r[None, :] <= r[:, None], 0.0, NEG).astype(np.float32)
    mb_g0 = np.full((P, P), NEG, dtype=np.float32)
    mb_g1 = np.zeros((P, P), dtype=np.float32)

    in_maps = []
    perms = []
    for core in range(8):
        b, g = core // 2, core % 2
        mine = list(range(g, NB, 2))
        partner = list(range(1 - g, NB, 2))
        perm = np.concatenate(
            [np.arange(blk * P, (blk + 1) * P) for blk in mine + partner])
        perms.append(perm)
        xt = np.ascontiguousarray(x[b][perm].T)  # [D, T] fp32
        xth, xtl = _split_bf(xt)
        in_maps.append({
            "xt_hi": xth, "xt_lo": xtl,
            "wq_hi": wqh, "wq_lo": wql, "wk_hi": wkh, "wk_lo": wkl,
            "wv": wvh, "bq8": bq8, "bkv": bkv,
            "maska": maska, "maskb": mb_g1 if g else mb_g0,
        })

    nc = _get_nc()
    res = run_bass_kernel_spmd(nc, in_maps, core_ids=list(range(8)),
                               trace=_trace, **_run_kwargs)
    LAST_RESULT = res

    out = np.zeros((B, T, H), dtype=np.float32)
    for core in range(8):
        b = core // 2
        out[b][perms[core][:NQ * P]] = res.results[core]["out"]
    return out



# revision 9
# speedup vs baseline: 1.1134x; 1.1134x over previous
"""Single-head causal attention (B=4, T=2048, D=1024, H=64) on 8 TRN2 cores.

Sharding: core = (batch b, group g). Each core owns the interleaved half of
the query blocks of one batch (g=0: even 128-row blocks, g=1: odd), arranged
"mine first, partner second" via a host-side row permutation so the causal
structure (and hence the instruction stream) is identical on all 8 cores.

v2 rewrite (tensor-cycle diet + HAM warmth):
- q and k projections packed into one [wq|wk] weight load -> 3 passes over
  full T cover BOTH q and k (49k cycles vs 61k separate).
- q/k biases added exactly via a K=2 matmul of (bias_hi;bias_lo) x ones.
- scores: q stacked as (qh;ql) on 128 partitions, k as (kh;kh) and (kl;kl):
  2 matmul passes compute the full (qh+ql)(kh+kl) product (vs 3 passes at
  64-contraction) with full PE utilization.
- the x8 score scale is folded into the softmax exp (scale=8, bias=-8*max).
- junk warm-up matmuls at t=0 trip the PE HAM clock gate (1.2->2.4 GHz)
  before the first x chunk lands; the c-loop keeps the PE busy thereafter.
- AV is interleaved with the descending scores loop (per-block start/stop
  bookkeeping) so the PE never drains between phases.
"""

import numpy as np
import ml_dtypes

import concourse.bass as bass
import concourse.bacc as bacc
import concourse.tile as tile
import concourse.mybir as mybir
from concourse.bass_utils import run_bass_kernel_spmd
from concourse.masks import make_identity

BF16 = mybir.dt.bfloat16
F32 = mybir.dt.float32
BFNP = ml_dtypes.bfloat16

B, T, D, H, P = 4, 2048, 1024, 64, 128
NB = T // P       # 16 key tiles (128 rows each) per batch
NQ = NB // 2      # 8 local query blocks per core
DC = D // P       # 8 contraction chunks
KC = 512          # matmul moving-dim chunk (one PSUM bank of fp32)
NEG = -1.0e30
ACT = mybir.ActivationFunctionType


def _bank_chunks(c0, c1):
    """Split [c0, c1) into <=512-wide pieces that don't cross 512 boundaries."""
    out = []
    while c0 < c1:
        nxt = min(c1, (c0 // KC + 1) * KC)
        out.append((c0, nxt))
        c0 = nxt
    return out


def build_nc():
    nc = bacc.Bacc("TRN2", target_bir_lowering=False, debug=False,
                   num_devices=8)
    xt_hi = nc.dram_tensor("xt_hi", [D, T], BF16, kind="ExternalInput")
    xt_lo = nc.dram_tensor("xt_lo", [D, T], BF16, kind="ExternalInput")
    # packed [wq|wk] weights, SBUF layout [P, DC*128] (chunk-major free dim)
    wqkh = nc.dram_tensor("wqkh", [P, DC * P], BF16, kind="ExternalInput")
    wqkl = nc.dram_tensor("wqkl", [P, DC * P], BF16, kind="ExternalInput")
    wv = nc.dram_tensor("wv", [P, DC * H], BF16, kind="ExternalInput")
    bias2 = nc.dram_tensor("bias2", [2, P], BF16, kind="ExternalInput")
    bv1 = nc.dram_tensor("bv1", [H, 1], F32, kind="ExternalInput")
    maska = nc.dram_tensor("maska", [P, P], F32, kind="ExternalInput")
    maskb = nc.dram_tensor("maskb", [P, P], F32, kind="ExternalInput")
    out = nc.dram_tensor("out", [NQ * P, H], F32, kind="ExternalOutput")

    with tile.TileContext(nc) as tc:
        with (
            tc.tile_pool(name="singles", bufs=1) as singles,
            tc.tile_pool(name="probs", bufs=3) as probs_pool,
            tc.tile_pool(name="stats", bufs=8) as stats,
        ):
            # ---- persistent SBUF ----
            s_xth = singles.tile([P, DC, T], BF16)
            s_xtl = singles.tile([P, DC, T], BF16)
            s_wqkh = singles.tile([P, DC, P], BF16)
            s_wqkl = singles.tile([P, DC, P], BF16)
            s_wv = singles.tile([P, DC, H], BF16)
            s_bias2 = singles.tile([2, P], BF16)
            s_ones = singles.tile([2, KC], BF16)
            s_bv = singles.tile([H, 1], F32)
            s_ma = singles.tile([P, P], F32)
            s_mb = singles.tile([P, P], F32)

            # weights/bias/masks first on the scalar queue (small, fast)
            nc.scalar.dma_start(s_bias2[:, :], bias2[:, :])
            for s_w, d_w in ((s_wv, wv), (s_wqkh, wqkh), (s_wqkl, wqkl)):
                nc.scalar.dma_start(
                    s_w[:, :, :].rearrange("p c h -> p (c h)"), d_w[:, :])
            nc.scalar.dma_start(s_bv[:, :], bv1[:, :])
            nc.scalar.dma_start(s_ma[:, :], maska[:, :])
            nc.scalar.dma_start(s_mb[:, :], maskb[:, :])
            nc.gpsimd.memset(s_ones[:, :], 1.0)
            # x chunks in consumption order: hi on sync, lo on scalar
            for c in range(DC):
                nc.sync.dma_start(s_xth[:, c, :], xt_hi[c * P:(c + 1) * P, :])
                nc.scalar.dma_start(s_xtl[:, c, :],
                                    xt_lo[c * P:(c + 1) * P, :])

            s_v = singles.tile([H, T], BF16)
            s_vnat = singles.tile([P, NB, H], BF16)
            s_qhl = singles.tile([P, NQ * P], BF16)   # rows 0:64 qh, 64:128 ql
            s_qlt = singles.tile([H, NQ * P], BF16)   # ql staging (parts 0:64)
            s_k1 = singles.tile([P, T], BF16)         # (kh;kh)
            s_k2 = singles.tile([P, T], BF16)         # (kl;kl)
            s_probsT = singles.tile([P, NB, NQ * P], BF16)
            s_z = singles.tile([P, NQ, 4], F32)
            nc.gpsimd.memset(s_z[:, :, :], 0.0)
            s_outT = singles.tile([H, NQ * P], F32)
            s_id = singles.tile([H, H], F32)
            make_identity(nc, s_id[:, :])

            # ---- phase 1: projections ----
            with tc.tile_pool(name="vp_psum", bufs=1, space="PSUM") as vp:
                vps = vp.tile([H, T], F32, tag="v")
                with tc.tile_pool(name="pj_psum", bufs=1, space="PSUM") as pj:
                    qkps = pj.tile([P, T], F32, tag="qk")
                    # HAM warm-up: junk K=2 matmuls (overwritten by the real
                    # group's start=True) keep the PE busy while x streams in.
                    for _ in range(4):
                        nc.tensor.matmul(qkps[:, 0:KC], lhsT=s_bias2[:, :],
                                         rhs=s_ones[:, :], start=True,
                                         stop=True, skip_group_check=True)
                    # exact q/k bias: (bias_hi;bias_lo)^T @ ones, K=2
                    for n0 in range(0, T, KC):
                        nc.tensor.matmul(qkps[:, n0:n0 + KC],
                                         lhsT=s_bias2[:, :],
                                         rhs=s_ones[:, :],
                                         start=True, stop=False)
                    passes = ((s_wqkh, s_xth), (s_wqkl, s_xth),
                              (s_wqkh, s_xtl))
                    for c in range(DC):
                        if c < DC - 2:
                            for n0 in range(0, T, KC):
                                nc.tensor.matmul(
                                    vps[:, n0:n0 + KC], lhsT=s_wv[:, c, :],
                                    rhs=s_xth[:, c, n0:n0 + KC],
                                    start=(c == 0), stop=False)
                        if c == DC - 1:
                            break
                        for w_pl, x_pl in passes:
                            for n0 in range(0, T, KC):
                                nc.tensor.matmul(
                                    qkps[:, n0:n0 + KC], lhsT=w_pl[:, c, :],
                                    rhs=x_pl[:, c, n0:n0 + KC],
                                    start=False, stop=False)

                    # peeled c=7 per 512-col region, bf16 hi/lo split fused
                    # right behind each region; order r0, r2 first so the
                    # small query blocks (0..3) unblock earliest.
                    c = DC - 1
                    for n0 in (0, 2 * KC, KC, 3 * KC):
                        for ip, (w_pl, x_pl) in enumerate(passes):
                            nc.tensor.matmul(
                                qkps[:, n0:n0 + KC], lhsT=w_pl[:, c, :],
                                rhs=x_pl[:, c, n0:n0 + KC],
                                start=False, stop=(ip == 2))
                        # k split: kh -> s_k1 rows 64:128 (lane-aligned),
                        # kl = kf - kh -> s_k2 rows 64:128
                        nc.scalar.copy(s_k1[H:P, n0:n0 + KC],
                                       qkps[H:P, n0:n0 + KC])
                        nc.vector.tensor_tensor(
                            s_k2[H:P, n0:n0 + KC], qkps[H:P, n0:n0 + KC],
                            s_k1[H:P, n0:n0 + KC], mybir.AluOpType.subtract)
                        # duplicate into rows 0:64 via SBUF->SBUF DMA
                        nc.gpsimd.dma_start(s_k1[0:H, n0:n0 + KC],
                                            s_k1[H:P, n0:n0 + KC])
                        nc.gpsimd.dma_start(s_k2[0:H, n0:n0 + KC],
                                            s_k2[H:P, n0:n0 + KC])
                        if n0 < NQ * P:  # q region: qh + ql staging
                            nc.scalar.copy(s_qhl[0:H, n0:n0 + KC],
                                           qkps[0:H, n0:n0 + KC])
                            nc.vector.tensor_tensor(
                                s_qlt[:, n0:n0 + KC], qkps[0:H, n0:n0 + KC],
                                s_qhl[0:H, n0:n0 + KC],
                                mybir.AluOpType.subtract)
                            nc.gpsimd.dma_start(s_qhl[H:P, n0:n0 + KC],
                                                s_qlt[:, n0:n0 + KC])

                # deferred v chunks cover the split/dup latency on the PE
                for c in (DC - 2, DC - 1):
                    for n0 in range(0, T, KC):
                        nc.tensor.matmul(
                            vps[:, n0:n0 + KC], lhsT=s_wv[:, c, :],
                            rhs=s_xth[:, c, n0:n0 + KC],
                            start=False, stop=(c == DC - 1))
                # v copy PSUM->SBUF pieces are interleaved with the first
                # scores blocks below so the DVE max-reduces aren't blocked.
                vcopies = [
                    lambda n0=n0: nc.vector.tensor_copy(
                        s_v[:, n0:n0 + KC], vps[:, n0:n0 + KC])
                    for n0 in range(0, T, KC)
                ]

                # -- phase 2: scores (vps still open: 4 + sc 4 = 8 banks) --
                with tc.tile_pool(name="sc_psum", bufs=4,
                                  space="PSUM") as sc:
                    for bi, i in enumerate((3, 2, 7, 6, 5, 4, 1, 0)):
                        if bi < len(vcopies):
                            vcopies[bi]()
                        if bi == len(vcopies):
                            nc.sync.dma_start(s_vnat[:, :, :], s_v[:, :],
                                              transpose=True)
                        K = P * (i + 1)
                        q0, q1 = i * P, (i + 1) * P
                        nch = (K + KC - 1) // KC  # chunks per part (1 or 2)
                        mx4 = stats.tile([P, 4], F32, tag="mx")
                        chunks = []
                        for pi, (koff, msk) in enumerate(((0, s_ma),
                                                          (NQ * P, s_mb))):
                            for n0 in range(0, K, KC):
                                nn = min(KC, K - n0)
                                sp = sc.tile([P, KC], F32, tag="sc")
                                nc.tensor.matmul(
                                    sp[:, 0:nn], lhsT=s_qhl[:, q0:q1],
                                    rhs=s_k1[:, koff + n0:koff + n0 + nn],
                                    start=True, stop=False)
                                nc.tensor.matmul(
                                    sp[:, 0:nn], lhsT=s_qhl[:, q0:q1],
                                    rhs=s_k2[:, koff + n0:koff + n0 + nn],
                                    start=False, stop=True)
                                if n0 + nn == K:  # mask in the last chunk
                                    nc.vector.tensor_tensor(
                                        sp[:, nn - P:nn], sp[:, nn - P:nn],
                                        msk[:, :], mybir.AluOpType.add)
                                ci = pi * nch + n0 // KC
                                nc.vector.tensor_reduce(
                                    mx4[:, ci:ci + 1], sp[:, 0:nn],
                                    axis=mybir.AxisListType.X,
                                    op=mybir.AluOpType.max)
                                chunks.append((sp, pi, n0, nn, ci))
                        mxc = stats.tile([P, 1], F32, tag="mxc")
                        nc.vector.tensor_reduce(mxc[:, :], mx4[:, 0:2 * nch],
                                                axis=mybir.AxisListType.X,
                                                op=mybir.AluOpType.max)
                        negm8 = stats.tile([P, 1], F32, tag="negm")
                        nc.vector.tensor_scalar_mul(negm8[:, :], mxc[:, :],
                                                    -8.0)
                        # exp(8*s - 8*max) with fused row-sum; x8 folds the
                        # reference's sqrt(head_size) score scale
                        probs = probs_pool.tile([P, T], BF16)
                        for sp, pi, n0, nn, ci in chunks:
                            po = pi * K + n0
                            nc.scalar.activation(
                                probs[:, po:po + nn], sp[:, 0:nn],
                                ACT.Exp, bias=negm8[:, :], scale=8.0,
                                accum_out=s_z[:, i, ci:ci + 1])
                        nc.sync.dma_start(s_probsT[:, 0:i + 1, q0:q1],
                                          probs[:, 0:K], transpose=True)
                        nc.scalar.dma_start(s_probsT[:, NQ:NQ + i + 1, q0:q1],
                                            probs[:, K:2 * K], transpose=True)

            # ---- phase 2b: AV (descending t so deps resolve in order) ----
            with (
                tc.tile_pool(name="av_psum", bufs=1, space="PSUM") as avp,
                tc.tile_pool(name="tr_psum", bufs=2, space="PSUM") as trp,
            ):
                # ascending t: t=0 fully initializes each PSUM bank
                # (start=True clears has_written at bank granularity, so the
                # first write into a bank must be the start of its group).
                avps = avp.tile([H, NQ * P], F32, tag="av")
                for t in range(NB):
                    j = t % NQ
                    c0 = j * P
                    if t < NQ:
                        chunks_av = [(a, b, False)
                                     for a, b in _bank_chunks(c0, NQ * P)]
                    else:
                        chunks_av = [(c0, c0 + P, True)]
                        chunks_av += [(a, b, False)
                                      for a, b in _bank_chunks(c0 + P, NQ * P)]
                    for a, b, stp in chunks_av:
                        nc.tensor.matmul(
                            avps[:, a:b], lhsT=s_vnat[:, t, :],
                            rhs=s_probsT[:, t, a:b], start=(t == 0), stop=stp)

                # per-block +bv copy out of PSUM so the finalize pipelines
                for j in range(NQ):
                    nc.scalar.activation(s_outT[:, j * P:(j + 1) * P],
                                         avps[:, j * P:(j + 1) * P],
                                         ACT.Identity, bias=s_bv[:, 0:1],
                                         scale=1.0)

                # ---- phase 3: transpose to [q, H], divide by Z, store ----
                s_zs = singles.tile([P, NQ], F32)
                nc.vector.tensor_reduce(s_zs[:, :], s_z[:, :, :],
                                        axis=mybir.AxisListType.X,
                                        op=mybir.AluOpType.add)
                s_rz = singles.tile([P, NQ], F32)
                nc.vector.reciprocal(s_rz[:, :], s_zs[:, :])
                s_on = singles.tile([P, NQ, H], F32)
                out3 = out[:, :].rearrange("(j p) h -> j p h", p=P)
                for j in range(NQ):
                    tps = trp.tile([P, H], F32, tag="tr")
                    nc.tensor.transpose(tps[:, :],
                                        s_outT[:, j * P:(j + 1) * P],
                                        s_id[:, :])
                    nc.vector.tensor_scalar_mul(s_on[:, j, :], tps[:, :],
                                                s_rz[:, j:j + 1])
                    nc.sync.dma_start(out3[j, :, :], s_on[:, j, :])
    nc.compile()
    return nc


_NC_CACHE = {}


def _get_nc():
    if "nc" not in _NC_CACHE:
        _NC_CACHE["nc"] = build_nc()
    return _NC_CACHE["nc"]


def _split_bf(a):
    hi = a.astype(BFNP)
    lo = (a - hi.astype(np.float32)).astype(BFNP)
    return hi, lo


def _w_layout(w, m):
    # [D, m] -> SBUF layout [P, DC*m] (chunk-major along free dim)
    return np.ascontiguousarray(
        w.reshape(DC, P, m).transpose(1, 0, 2).reshape(P, DC * m))


LAST_RESULT = None


def kernel(x, Wq, bq, Wk, bk, Wv, bv, _trace=False, **_run_kwargs):
    global LAST_RESULT
    x = np.ascontiguousarray(np.asarray(x, dtype=np.float32))
    Wq = np.asarray(Wq, dtype=np.float32)
    Wk = np.asarray(Wk, dtype=np.float32)
    Wv = np.asarray(Wv, dtype=np.float32)
    bq = np.asarray(bq, dtype=np.float32)
    bk = np.asarray(bk, dtype=np.float32)
    bv = np.asarray(bv, dtype=np.float32)

    wqk = np.concatenate([Wq, Wk], axis=1)  # [D, 128]
    wqk_hi, wqk_lo = _split_bf(wqk)
    wqkh, wqkl = _w_layout(wqk_hi, P), _w_layout(wqk_lo, P)
    wvh = _w_layout(Wv.astype(BFNP), H)
    bqk = np.concatenate([bq, bk])  # [128]
    b_hi = bqk.astype(BFNP)
    b_lo = (bqk - b_hi.astype(np.float32)).astype(BFNP)
    bias2 = np.ascontiguousarray(np.stack([b_hi, b_lo]))  # [2, 128] bf16
    bv1 = np.ascontiguousarray(bv.reshape(H, 1))
    r = np.arange(P)
    maska = np.where(r[None, :] <= r[:, None], 0.0, NEG).astype(np.float32)
    mb_g0 = np.full((P, P), NEG, dtype=np.float32)
    mb_g1 = np.zeros((P, P), dtype=np.float32)

    in_maps = []
    perms = []
    for core in range(8):
        b, g = core // 2, core % 2
        mine = list(range(g, NB, 2))
        partner = list(range(1 - g, NB, 2))
        perm = np.concatenate(
            [np.arange(blk * P, (blk + 1) * P) for blk in mine + partner])
        perms.append(perm)
        xt = np.ascontiguousarray(x[b][perm].T)  # [D, T] fp32
        xth, xtl = _split_bf(xt)
        in_maps.append({
            "xt_hi": xth, "xt_lo": xtl,
            "wqkh": wqkh, "wqkl": wqkl, "wv": wvh,
            "bias2": bias2, "bv1": bv1,
            "maska": maska, "maskb": mb_g1 if g else mb_g0,
        })

    nc = _get_nc()
    res = run_bass_kernel_spmd(nc, in_maps, core_ids=list(range(8)),
                               trace=_trace, **_run_kwargs)
    LAST_RESULT = res

    out = np.zeros((B, T, H), dtype=np.float32)
    for core in range(8):
        b = core // 2
        out[b][perms[core][:NQ * P]] = res.results[core]["out"]
    return out
